# revision 13
# baseline (speedup 1.0000x reference)
"""Trainium2 Bass kernel for nn_CausalBiBCNAttention (B=4, T=4096, D=1024, R=256).

Algebra (exact rewrite of the reference):
    out = G @ (Wo@U).T + min(n,1)*(1+alpha)*(Wo@bias)
    G   = (A*cumsum(Bk) + E*cumsum(C)) / max(n,1)
    A   = x @ (Wq.T V);  E = x @ (Wq.T Winv.T Wm)
    Bk  = (x @ (Wk.T Wm)) * m;  C = alpha * (x @ (Wk.T Winv.T V)) * m
    n   = cumsum(m)
The five DxD projections fold into four DxR matrices (host constant folding in
f64); the device does 5 rank-R projections + DVE prefix-scans (cumsum maps to
the native tensor_tensor_scan along the free axis).

Precision: matmul operands are fp16 hi/lo pairs (x = xh + xl exactly to
~2^-22); each contraction runs 3 passes (xh*Ph + xh*Pl + xl*Ph, the xl*Pl
term is ~2^-22 and dropped) accumulated in fp32 PSUM. fp16 streams at
1 col/cycle on the PE (fp32 runs 2 half-rate passes = 4x slower).

Sharding: 8 cores = batch(4) x sequence-halves(2). The cumsum carry for the
second half is computed on-device from a mask-weighted reduction of the
previous half (xbar = mprev^T @ xprev; S = xbar @ [P3|P4]).
"""

from contextlib import ExitStack

import numpy as np

import concourse.bass as bass
import concourse.mybir as mybir
import concourse.tile as tile
from concourse.bass_utils import run_bass_kernel_spmd

F32 = mybir.dt.float32
F16 = mybir.dt.float16
AL = mybir.AluOpType

N_CORES = 8
N_SEQ_SHARDS = 2


def _split16(a):
    """fp16 hi/lo pair: a ~= hi + lo with ~2^-22 relative residual."""
    hi = a.astype(np.float16)
    lo = (a - hi.astype(np.float32)).astype(np.float16)
    return hi, lo


def fold_weights(Wq, Wk, Wo, Winv, U, V, Wm, bias, alpha):
    Wq, Wk, Wo, Winv, U, V, Wm, bias = (
        np.asarray(a, np.float64) for a in (Wq, Wk, Wo, Winv, U, V, Wm, bias)
    )
    alpha = float(alpha)
    P1 = Wq.T @ V
    P2 = Wq.T @ Winv.T @ Wm
    P3 = Wk.T @ Wm
    P4 = alpha * (Wk.T @ (Winv.T @ V))
    Pcat = np.concatenate([P1, P2, P3, P4], axis=1).astype(np.float32)
    ZT = np.ascontiguousarray((Wo @ U).T).astype(np.float32)
    bvec = ((1.0 + alpha) * (Wo @ bias)).astype(np.float32)[None, :]
    return Pcat, ZT, bvec


def split_excess_waits(nc, max_waits=1):
    """Hoist excess per-instruction sync waits onto preceding same-engine NoOps.

    Walrus's per-instruction sync budget rejects >1 wait command on several
    instruction structs (fp32 Matmult, DMA pseudo-ops). Engine streams execute
    in order, so a NoOp carrying the extra wait immediately before the
    instruction is semantically identical.
    """
    fn = nc.m.functions[0]
    k = 0
    for blk in fn.blocks:
        new_insts = []
        for ins in blk.instructions:
            si = getattr(ins, "sync_info", None)
            if si is not None and si.on_wait and len(si.on_wait) > max_waits:
                waits = list(si.on_wait)
                for w in waits[:-max_waits]:
                    k += 1
                    new_insts.append(
                        mybir.InstNoOp(
                            name=f"{ins.name}-hoistw{k}",
                            engine=ins.engine,
                            ins=[],
                            outs=[],
                            sync_info=mybir.SyncInfo(on_wait=[w], on_update=[]),
                            bass_nofuse=True,
                        )
                    )
                ins.sync_info = mybir.SyncInfo(
                    on_wait=waits[-max_waits:], on_update=si.on_update
                )
            new_insts.append(ins)
        blk.instructions[:] = new_insts
    return nc


def build_nc(D, TC, R, TT=512, hoist=True):
    assert D % 128 == 0 and R % 128 == 0 and TC % TT == 0 and TC % 128 == 0
    nd, nr, nt, ntc = D // 128, R // 128, TC // TT, TC // 128

    nc = bass.Bass()
    xTh = nc.dram_tensor("xTh", (D, TC), F16, kind="ExternalInput")
    xTl = nc.dram_tensor("xTl", (D, TC), F16, kind="ExternalInput")
    xprevh = nc.dram_tensor("xprevh", (TC, D), F16, kind="ExternalInput")
    xprevl = nc.dram_tensor("xprevl", (TC, D), F16, kind="ExternalInput")
    mrow = nc.dram_tensor("mrow", (1, TC), F16, kind="ExternalInput")
    mprev = nc.dram_tensor("mprev", (128, ntc), F16, kind="ExternalInput")
    Pcath = nc.dram_tensor("Pcath", (D, 4 * R), F16, kind="ExternalInput")
    Pcatl = nc.dram_tensor("Pcatl", (D, 4 * R), F16, kind="ExternalInput")
    ZTh = nc.dram_tensor("ZTh", (R, D), F16, kind="ExternalInput")
    ZTl = nc.dram_tensor("ZTl", (R, D), F16, kind="ExternalInput")
    bvecd = nc.dram_tensor("bvec", (1, D), F16, kind="ExternalInput")
    outT = nc.dram_tensor("outT", (D, TC), F32, kind="ExternalOutput")

    with tile.TileContext(nc) as tc, ExitStack() as ctx:
        res = ctx.enter_context(tc.tile_pool(name="res", bufs=1))
        psb = ctx.enter_context(tc.tile_pool(name="psb", bufs=4, space="PSUM"))
        pss = ctx.enter_context(tc.tile_pool(name="pss", bufs=3, space="PSUM"))
        psj = ctx.enter_context(tc.tile_pool(name="psj", bufs=1, space="PSUM"))
        junk = psj.tile([1, 1], F32, tag="junk", name="junk")

        def touch(t):
            # absorb the tile's DMA-completion wait into a 1-element PE matmul
            # (several instruction structs carry at most ONE sync wait; this
            # keeps every real matmul's unsatisfied-dependency count at <= 1)
            k = min(t.shape[0], 128)
            a = t[0:k, 0:1]
            nc.tensor.matmul(junk[:, :], a, a, start=True, stop=True)

        # --- resident tiles ---
        xtsh = [res.tile([128, TC], F16, tag=f"xth{d}", name=f"xth{d}") for d in range(nd)]
        xtsl = [res.tile([128, TC], F16, tag=f"xtl{d}", name=f"xtl{d}") for d in range(nd)]
        pcsh = [res.tile([128, 4 * R], F16, tag=f"pch{d}", name=f"pch{d}") for d in range(nd)]
        pcsl = [res.tile([128, 4 * R], F16, tag=f"pcl{d}", name=f"pcl{d}") for d in range(nd)]
        ztsh = [res.tile([128, D], F16, tag=f"zth{r}", name=f"zth{r}") for r in range(nr)]
        ztsl = [res.tile([128, D], F16, tag=f"ztl{r}", name=f"ztl{r}") for r in range(nr)]
        cums = [res.tile([128, TC], F32, tag=f"cum{q}", name=f"cum{q}") for q in range(2 * nr)]
        nb = res.tile([128, TC], F32, tag="nb", name="nb")
        minn = res.tile([1, TC], F16, tag="minn", name="minn")
        bvec_sb = res.tile([1, D], F16, tag="bvec", name="bvec")

        mrow_pre = res.tile([1, TC], F16, tag="mrowp", name="mrowp")
        mprev_pre = res.tile([128, ntc], F16, tag="mprevp", name="mprevp")
        nc.sync.dma_start(mrow_pre[:, :], mrow[:, :])
        touch(mrow_pre)
        nc.sync.dma_start(mprev_pre[:, :], mprev[:, :])
        touch(mprev_pre)
        for d in range(nd):
            nc.sync.dma_start(pcsh[d][:, :], Pcath[d * 128 : (d + 1) * 128, :])
            touch(pcsh[d])
            nc.sync.dma_start(xtsh[d][:, :], xTh[d * 128 : (d + 1) * 128, :])
            touch(xtsh[d])
        for d in range(nd):
            nc.sync.dma_start(pcsl[d][:, :], Pcatl[d * 128 : (d + 1) * 128, :])
            touch(pcsl[d])
            nc.sync.dma_start(xtsl[d][:, :], xTl[d * 128 : (d + 1) * 128, :])
            touch(xtsl[d])
        for r in range(nr):
            nc.sync.dma_start(ztsh[r][:, :], ZTh[r * 128 : (r + 1) * 128, :])
            touch(ztsh[r])
            nc.sync.dma_start(ztsl[r][:, :], ZTl[r * 128 : (r + 1) * 128, :])
            touch(ztsl[r])
        nc.sync.dma_start(bvec_sb[:, :], bvecd[:, :])
        touch(bvec_sb)

        with ExitStack() as ectx:
            early = ectx.enter_context(tc.tile_pool(name="early", bufs=1))
            xpp = ectx.enter_context(tc.tile_pool(name="xpp", bufs=2))
            bkp = ectx.enter_context(tc.tile_pool(name="bkp", bufs=4))

            masks = early.tile([128, TC], F16, tag="masks", name="masks")
            ones_col = early.tile([128, 1], F16, tag="ones_col", name="ones_col")
            ones_row = early.tile([1, 128], F16, tag="ones_row", name="ones_row")
            id1 = early.tile([1, 1], F32, tag="id1", name="id1")
            xbar_sb = early.tile([1, D], F32, tag="xbar", name="xbar")
            xbar_fm = early.tile([128, nd], F32, tag="xbarfm", name="xbarfm")
            xfh = early.tile([128, nd], F16, tag="xfh", name="xfh")
            xfh32 = early.tile([128, nd], F32, tag="xfh32", name="xfh32")
            xfl = early.tile([128, nd], F16, tag="xfl", name="xfl")
            S_sb = early.tile([1, 2 * R], F32, tag="Ssb", name="Ssb")
            noff_sb = early.tile([1, 1], F16, tag="noff", name="noff")
            noffb = early.tile([128, 1], F32, tag="noffb", name="noffb")
            inits = [
                early.tile([128, 1], F32, tag=f"init{q}", name=f"init{q}")
                for q in range(2 * nr)
            ]

            nc.vector.memset(ones_col[:, :], 1.0)
            nc.vector.memset(ones_row[:, :], 1.0)
            nc.vector.memset(id1[:, :], 1.0)

            # mask broadcast (rank-1 PE outer product), resident [128, TC] f16
            for t in range(nt):
                tsl = slice(t * TT, (t + 1) * TT)
                psm = pss.tile([128, TT], F32, tag="small", name="small")
                nc.tensor.matmul(
                    psm[:, :], ones_row[:, :], mrow_pre[:, tsl], start=True, stop=True
                )
                nc.vector.tensor_copy(masks[:, tsl], psm[:, :])

            # n carry from mprev alone (no xprev dependency), n-scan + scalers
            nred = early.tile([128, 1], F32, tag="nred", name="nred")
            nred16 = early.tile([128, 1], F16, tag="nred16", name="nred16")
            nc.vector.tensor_reduce(nred[:, :], mprev_pre[:, :], mybir.AxisListType.X, AL.add)
            nc.vector.tensor_copy(nred16[:, :], nred[:, :])
            ps_nf = pss.tile([1, 1], F32, tag="small", name="small")
            nc.tensor.matmul(ps_nf[:, :], nred16[:, :], ones_col[:, :], start=True, stop=True)
            nc.vector.tensor_copy(noff_sb[:, :], ps_nf[:, :])
            ps_nb = pss.tile([128, 1], F32, tag="small", name="small")
            nc.tensor.matmul(ps_nb[:, :], ones_row[:, :], noff_sb[:, :], start=True, stop=True)
            nc.vector.tensor_copy(noffb[:, :], ps_nb[:, :])
            for t in range(nt):
                tsl = slice(t * TT, (t + 1) * TT)
                init = noffb[:, :] if t == 0 else nb[:, t * TT - 1 : t * TT]
                nc.vector.tensor_tensor_scan(
                    nb[:, tsl], masks[:, tsl], masks[:, tsl], init, AL.add, AL.bypass
                )
            nc.vector.tensor_scalar_min(minn[:, :], nb[0:1, :], 1.0)
            for t in range(nt):
                tsl = slice(t * TT, (t + 1) * TT)
                nc.vector.tensor_scalar_max(nb[:, tsl], nb[:, tsl], 1.0)
                nc.vector.reciprocal(nb[:, tsl], nb[:, tsl])

            # K-side projections (Bk, C): 3-pass fp16 hi/lo + masked evac
            bks = {}
            for q in range(2 * nr):
                mcol = 2 * R + q * 128
                for t in range(nt):
                    tsl = slice(t * TT, (t + 1) * TT)
                    pt = psb.tile([128, TT], F32, tag="pt", name="pt")
                    first = True
                    for xs, ps in ((xtsh, pcsh), (xtsh, pcsl), (xtsl, pcsh)):
                        for d in range(nd):
                            nc.tensor.matmul(
                                pt[:, :], ps[d][:, mcol : mcol + 128], xs[d][:, tsl],
                                start=first,
                                stop=(xs is xtsl and d == nd - 1),
                            )
                            first = False
                    bk = bkp.tile([128, TT], F32, tag="bk", name="bk")
                    nc.vector.tensor_mul(bk[:, :], pt[:, :], masks[:, tsl])
                    bks[(q, t)] = bk

            # cross-half carry: xbar = mprev^T @ (xprevh + xprevl)
            n512 = (D + 511) // 512
            ps_xb = [
                pss.tile([1, min(512, D - j * 512)], F32, tag="small", name="small")
                for j in range(n512)
            ]
            assert ntc % 2 == 0
            for half, xsrc in enumerate((xprevh, xprevl)):
                for i2 in range(ntc // 2):
                    xp = xpp.tile([128, 2 * D], F16, tag="xprev", name="xprev")
                    src_ap = xsrc[i2 * 256 : (i2 + 1) * 256, :].rearrange(
                        "(c p) d -> p c d", p=128
                    )
                    nc.sync.dma_start(
                        xp[:, :].rearrange("p (c d) -> p c d", c=2), src_ap
                    )
                    touch(xp)
                    for c in range(2):
                        i = 2 * i2 + c
                        lhs = mprev_pre[:, i : i + 1]
                        for j in range(n512):
                            w = min(512, D - j * 512)
                            nc.tensor.matmul(
                                ps_xb[j][:, :], lhs,
                                xp[:, c * D + j * 512 : c * D + j * 512 + w],
                                start=(i == 0 and half == 0),
                                stop=(i == ntc - 1 and half == 1),
                            )
            for j in range(n512):
                w = min(512, D - j * 512)
                nc.vector.tensor_copy(xbar_sb[:, j * 512 : j * 512 + w], ps_xb[j][:, :])

            for j in range(nd):
                pst = pss.tile([128, 1], F32, tag="small", name="small")
                nc.tensor.transpose(
                    pst[:, :], xbar_sb[:, j * 128 : (j + 1) * 128], id1[:, :]
                )
                nc.vector.tensor_copy(xbar_fm[:, j : j + 1], pst[:, :])

            # split xbar into fp16 hi/lo, then S = xbar @ [P3|P4] (3-pass)
            nc.vector.tensor_copy(xfh[:, :], xbar_fm[:, :])
            nc.vector.tensor_copy(xfh32[:, :], xfh[:, :])
            nc.vector.tensor_sub(xfl[:, :], xbar_fm[:, :], xfh32[:, :])
            ps_S = pss.tile([1, 2 * R], F32, tag="small", name="small")
            for d in range(nd):
                ops = [(xfh, pcsh[d]), (xfh, pcsl[d]), (xfl, pcsh[d])]
                for k, (xo, po) in enumerate(ops):
                    nc.tensor.matmul(
                        ps_S[:, :], xo[:, d : d + 1], po[:, 2 * R : 4 * R],
                        start=(d == 0 and k == 0),
                        stop=(d == nd - 1 and k == len(ops) - 1),
                    )
            nc.vector.tensor_copy(S_sb[:, :], ps_S[:, :])

            for q in range(2 * nr):
                pst = pss.tile([128, 1], F32, tag="small", name="small")
                nc.tensor.transpose(
                    pst[:, :], S_sb[:, q * 128 : (q + 1) * 128], id1[:, :]
                )
                nc.vector.tensor_copy(inits[q][:, :], pst[:, :])

            # scans (cumsum along free axis, carry chained across t-tiles)
            for q in range(2 * nr):
                for t in range(nt):
                    tsl = slice(t * TT, (t + 1) * TT)
                    init = inits[q][:, :] if t == 0 else cums[q][:, t * TT - 1 : t * TT]
                    bk = bks[(q, t)]
                    nc.vector.tensor_tensor_scan(
                        cums[q][:, tsl], bk[:, :], bk[:, :], init, AL.add, AL.bypass
                    )

        # --- phase D: A/E projections, G, final matmul ---
        with ExitStack() as lctx:
            late = lctx.enter_context(tc.tile_pool(name="late", bufs=1))
            aep = lctx.enter_context(tc.tile_pool(name="aep", bufs=6))
            gp = lctx.enter_context(tc.tile_pool(name="gp", bufs=2))
            outp = lctx.enter_context(tc.tile_pool(name="outp", bufs=4))


            for t in range(nt):
                tsl = slice(t * TT, (t + 1) * TT)
                aes = []
                for m in range(2 * nr):
                    pa = psb.tile([128, TT], F32, tag="pt", name="pt")
                    first = True
                    for xs, ps in ((xtsh, pcsh), (xtsh, pcsl), (xtsl, pcsh)):
                        for d in range(nd):
                            nc.tensor.matmul(
                                pa[:, :], ps[d][:, m * 128 : (m + 1) * 128], xs[d][:, tsl],
                                start=first,
                                stop=(xs is xtsl and d == nd - 1),
                            )
                            first = False
                    ae = aep.tile([128, TT], F32, tag="ae", name="ae")
                    nc.scalar.copy(ae[:, :], pa[:, :])
                    aes.append(ae)
                ghs, gls = [], []
                for r in range(nr):
                    t1 = gp.tile([128, TT], F32, tag="g1", name="g1")
                    nc.vector.tensor_mul(t1[:, :], aes[r][:, :], cums[r][:, tsl])
                    t2 = gp.tile([128, TT], F32, tag="g2", name="g2")
                    nc.vector.tensor_mul(t2[:, :], aes[nr + r][:, :], cums[nr + r][:, tsl])
                    nc.vector.tensor_add(t1[:, :], t1[:, :], t2[:, :])
                    g = gp.tile([128, TT], F32, tag="g", name="g", bufs=4)
                    nc.vector.tensor_mul(g[:, :], t1[:, :], nb[:, tsl])
                    gh = gp.tile([128, TT], F16, tag="gh", name="gh", bufs=4)
                    nc.scalar.copy(gh[:, :], g[:, :])
                    gl = gp.tile([128, TT], F16, tag="gl", name="gl", bufs=4)
                    nc.vector.tensor_sub(gl[:, :], g[:, :], gh[:, :])
                    ghs.append(gh)
                    gls.append(gl)
                for d in range(nd):
                    po = psb.tile([128, TT], F32, tag="pt", name="pt")
                    dsl = slice(d * 128, (d + 1) * 128)
                    first = True
                    for r in range(nr):
                        for zo, go in (
                            (ztsh[r], ghs[r]),
                            (ztsl[r], ghs[r]),
                            (ztsh[r], gls[r]),
                        ):
                            nc.tensor.matmul(
                                po[:, :], zo[:, dsl], go[:, :], start=first, stop=False
                            )
                            first = False
                    nc.tensor.matmul(
                        po[:, :], bvec_sb[:, d * 128 : (d + 1) * 128], minn[:, tsl],
                        start=False, stop=True,
                    )
                    ot = outp.tile([128, TT], F32, tag="ot", name="ot")
                    nc.scalar.copy(ot[:, :], po[:, :])
                    nc.sync.dma_start(outT[d * 128 : (d + 1) * 128, tsl], ot[:, :])

    nc.finalize()
    if hoist:
        split_excess_waits(nc)
    return nc


def make_core_inputs(x, attention_mask, Pcat, ZT, bvec):
    B, T, D = x.shape
    TC = T // N_SEQ_SHARDS
    m = np.asarray(attention_mask).astype(np.float16)
    Ph, Pl = _split16(Pcat)
    Zh, Zl = _split16(ZT)
    bv16 = bvec.astype(np.float16)
    in_maps = []
    for b in range(B):
        for h in range(N_SEQ_SHARDS):
            sl = slice(h * TC, (h + 1) * TC)
            psl = slice((h - 1) * TC, h * TC) if h > 0 else slice(0, TC)
            mp = m[b, psl] if h > 0 else np.zeros(TC, np.float16)
            xT = np.ascontiguousarray(x[b, sl, :].T)
            xTh, xTl = _split16(xT)
            xprevh, xprevl = _split16(x[b, psl, :])
            in_maps.append(
                {
                    "xTh": xTh,
                    "xTl": xTl,
                    "xprevh": np.ascontiguousarray(xprevh),
                    "xprevl": np.ascontiguousarray(xprevl),
                    "mrow": np.ascontiguousarray(m[b, sl])[None, :],
                    "mprev": np.ascontiguousarray(mp.reshape(TC // 128, 128).T),
                    "Pcath": Ph,
                    "Pcatl": Pl,
                    "ZTh": Zh,
                    "ZTl": Zl,
                    "bvec": bv16,
                }
            )
    return in_maps


_NC_CACHE = {}


def get_nc(D, TC, R):
    key = (D, TC, R)
    if key not in _NC_CACHE:
        _NC_CACHE[key] = build_nc(D, TC, R)
    return _NC_CACHE[key]


def kernel(x, Wq, Wk, Wo, Winv, U, V, Wm, bias, alpha, attention_mask):
    x = np.asarray(x, np.float32)
    B, T, D = x.shape
    R = np.asarray(U).shape[1]
    TC = T // N_SEQ_SHARDS
    Pcat, ZT, bvec = fold_weights(Wq, Wk, Wo, Winv, U, V, Wm, bias, alpha)
    nc = get_nc(D, TC, R)
    in_maps = make_core_inputs(x, np.asarray(attention_mask), Pcat, ZT, bvec)
    res = run_bass_kernel_spmd(nc, in_maps, core_ids=list(range(N_CORES)))
    out = np.empty((B, T, D), np.float32)
    k = 0
    for b in range(B):
        for h in range(N_SEQ_SHARDS):
            out[b, h * TC : (h + 1) * TC, :] = res.results[k]["outT"].T
            k += 1
    return out


# revision 16
# speedup vs baseline: 1.0523x; 1.0523x over previous
"""Trainium2 Bass kernel for nn_CausalBiBCNAttention (B=4, T=4096, D=1024, R=256).

Algebra (exact rewrite of the reference):
    out = G @ (Wo@U).T + min(n,1)*(1+alpha)*(Wo@bias)
    G   = (A*cumsum(Bk) + E*cumsum(C)) / max(n,1)
    A   = x @ (Wq.T V);  E = x @ (Wq.T Winv.T Wm)
    Bk  = (x @ (Wk.T Wm)) * m;  C = alpha * (x @ (Wk.T Winv.T V)) * m
    n   = cumsum(m)
The five DxD projections fold into four DxR matrices (host constant folding in
f64); the device does 5 rank-R projections + DVE prefix-scans (cumsum maps to
the native tensor_tensor_scan along the free axis).

Precision: matmul operands are fp16 hi/lo pairs (x = xh + xl exactly to
~2^-22); each contraction runs 3 passes (xh*Ph + xh*Pl + xl*Ph, the xl*Pl
term is ~2^-22 and dropped) accumulated in fp32 PSUM. fp16 streams at
1 col/cycle on the PE (fp32 runs 2 half-rate passes = 4x slower).

Sharding: 8 cores = batch(4) x sequence-halves(2). The cumsum carry for the
second half is computed on-device from a mask-weighted reduction of the
previous half (xbar = mprev^T @ xprev; S = xbar @ [P3|P4]).
"""

from contextlib import ExitStack

import numpy as np

import concourse.bass as bass
import concourse.mybir as mybir
import concourse.tile as tile
from concourse.bass_utils import run_bass_kernel_spmd

F32 = mybir.dt.float32
F16 = mybir.dt.float16
AL = mybir.AluOpType

N_CORES = 8
N_SEQ_SHARDS = 2


def _split16(a):
    """fp16 hi/lo pair: a ~= hi + lo with ~2^-22 relative residual."""
    hi = a.astype(np.float16)
    lo = (a - hi.astype(np.float32)).astype(np.float16)
    return hi, lo


def fold_weights(Wq, Wk, Wo, Winv, U, V, Wm, bias, alpha):
    Wq, Wk, Wo, Winv, U, V, Wm, bias = (
        np.asarray(a, np.float64) for a in (Wq, Wk, Wo, Winv, U, V, Wm, bias)
    )
    alpha = float(alpha)
    P1 = Wq.T @ V
    P2 = Wq.T @ Winv.T @ Wm
    P3 = Wk.T @ Wm
    P4 = alpha * (Wk.T @ (Winv.T @ V))
    Pcat = np.concatenate([P1, P2, P3, P4], axis=1).astype(np.float32)
    ZT = np.ascontiguousarray((Wo @ U).T).astype(np.float32)
    bvec = ((1.0 + alpha) * (Wo @ bias)).astype(np.float32)[None, :]
    return Pcat, ZT, bvec


def split_excess_waits(nc, max_waits=1):
    """Hoist excess per-instruction sync waits onto preceding same-engine NoOps.

    Walrus's per-instruction sync budget rejects >1 wait command on several
    instruction structs (fp32 Matmult, DMA pseudo-ops). Engine streams execute
    in order, so a NoOp carrying the extra wait immediately before the
    instruction is semantically identical.
    """
    fn = nc.m.functions[0]
    k = 0
    for blk in fn.blocks:
        new_insts = []
        for ins in blk.instructions:
            si = getattr(ins, "sync_info", None)
            if si is not None and si.on_wait and len(si.on_wait) > max_waits:
                waits = list(si.on_wait)
                for w in waits[:-max_waits]:
                    k += 1
                    new_insts.append(
                        mybir.InstNoOp(
                            name=f"{ins.name}-hoistw{k}",
                            engine=ins.engine,
                            ins=[],
                            outs=[],
                            sync_info=mybir.SyncInfo(on_wait=[w], on_update=[]),
                            bass_nofuse=True,
                        )
                    )
                ins.sync_info = mybir.SyncInfo(
                    on_wait=waits[-max_waits:], on_update=si.on_update
                )
            new_insts.append(ins)
        blk.instructions[:] = new_insts
    return nc


def build_nc(D, TC, R, TT=512, hoist=True):
    assert D % 128 == 0 and R % 128 == 0 and TC % TT == 0 and TC % 128 == 0
    nd, nr, nt, ntc = D // 128, R // 128, TC // TT, TC // 128

    nc = bass.Bass()
    xTh = nc.dram_tensor("xTh", (D, TC), F16, kind="ExternalInput")
    xTl = nc.dram_tensor("xTl", (D, TC), F16, kind="ExternalInput")
    xprevh = nc.dram_tensor("xprevh", (TC, D), F16, kind="ExternalInput")
    xprevl = nc.dram_tensor("xprevl", (TC, D), F16, kind="ExternalInput")
    mrow = nc.dram_tensor("mrow", (1, TC), F16, kind="ExternalInput")
    mprev = nc.dram_tensor("mprev", (128, ntc), F16, kind="ExternalInput")
    Pcath = nc.dram_tensor("Pcath", (D, 4 * R), F16, kind="ExternalInput")
    Pcatl = nc.dram_tensor("Pcatl", (D, 4 * R), F16, kind="ExternalInput")
    ZTh = nc.dram_tensor("ZTh", (R, D), F16, kind="ExternalInput")
    ZTl = nc.dram_tensor("ZTl", (R, D), F16, kind="ExternalInput")
    bvecd = nc.dram_tensor("bvec", (1, D), F16, kind="ExternalInput")
    outT = nc.dram_tensor("outT", (D, TC), F32, kind="ExternalOutput")

    with tile.TileContext(nc) as tc, ExitStack() as ctx:
        res = ctx.enter_context(tc.tile_pool(name="res", bufs=1))
        psb = ctx.enter_context(tc.tile_pool(name="psb", bufs=4, space="PSUM"))
        pss = ctx.enter_context(tc.tile_pool(name="pss", bufs=3, space="PSUM"))
        psj = ctx.enter_context(tc.tile_pool(name="psj", bufs=1, space="PSUM"))
        junk = psj.tile([1, 1], F32, tag="junk", name="junk")

        def touch(t):
            # absorb the tile's DMA-completion wait into a 1-element PE matmul
            # (several instruction structs carry at most ONE sync wait; this
            # keeps every real matmul's unsatisfied-dependency count at <= 1)
            k = min(t.shape[0], 128)
            a = t[0:k, 0:1]
            nc.tensor.matmul(junk[:, :], a, a, start=True, stop=True)

        # --- resident tiles ---
        xtsh = [res.tile([128, TC], F16, tag=f"xth{d}", name=f"xth{d}") for d in range(nd)]
        xtsl = [res.tile([128, TC], F16, tag=f"xtl{d}", name=f"xtl{d}") for d in range(nd)]
        pcsh = [res.tile([128, 4 * R], F16, tag=f"pch{d}", name=f"pch{d}") for d in range(nd)]
        pcsl = [res.tile([128, 4 * R], F16, tag=f"pcl{d}", name=f"pcl{d}") for d in range(nd)]
        ztsh = [res.tile([128, D], F16, tag=f"zth{r}", name=f"zth{r}") for r in range(nr)]
        ztsl = [res.tile([128, D], F16, tag=f"ztl{r}", name=f"ztl{r}") for r in range(nr)]
        cums = [res.tile([128, TC], F32, tag=f"cum{q}", name=f"cum{q}") for q in range(2 * nr)]
        nb = res.tile([128, TC], F32, tag="nb", name="nb")
        minn = res.tile([1, TC], F16, tag="minn", name="minn")
        bvec_sb = res.tile([1, D], F16, tag="bvec", name="bvec")

        mrow_pre = res.tile([1, TC], F16, tag="mrowp", name="mrowp")
        mprev_pre = res.tile([128, ntc], F16, tag="mprevp", name="mprevp")
        nc.sync.dma_start(mrow_pre[:, :], mrow[:, :])
        touch(mrow_pre)
        nc.sync.dma_start(mprev_pre[:, :], mprev[:, :])
        touch(mprev_pre)
        for d in range(nd):
            nc.sync.dma_start(pcsh[d][:, :], Pcath[d * 128 : (d + 1) * 128, :])
            touch(pcsh[d])
            nc.sync.dma_start(xtsh[d][:, :], xTh[d * 128 : (d + 1) * 128, :])
            touch(xtsh[d])
        for d in range(nd):
            nc.sync.dma_start(pcsl[d][:, :], Pcatl[d * 128 : (d + 1) * 128, :])
            touch(pcsl[d])
            nc.sync.dma_start(xtsl[d][:, :], xTl[d * 128 : (d + 1) * 128, :])
            touch(xtsl[d])
        for r in range(nr):
            nc.sync.dma_start(ztsh[r][:, :], ZTh[r * 128 : (r + 1) * 128, :])
            touch(ztsh[r])
            nc.sync.dma_start(ztsl[r][:, :], ZTl[r * 128 : (r + 1) * 128, :])
            touch(ztsl[r])
        nc.sync.dma_start(bvec_sb[:, :], bvecd[:, :])
        touch(bvec_sb)

        with ExitStack() as ectx:
            early = ectx.enter_context(tc.tile_pool(name="early", bufs=1))
            xpp = ectx.enter_context(tc.tile_pool(name="xpp", bufs=2))
            bkp = ectx.enter_context(tc.tile_pool(name="bkp", bufs=4))

            masks = early.tile([128, TC], F16, tag="masks", name="masks")
            ones_col = early.tile([128, 1], F16, tag="ones_col", name="ones_col")
            ones_row = early.tile([1, 128], F16, tag="ones_row", name="ones_row")
            id1 = early.tile([1, 1], F32, tag="id1", name="id1")
            xbar_sb = early.tile([1, D], F32, tag="xbar", name="xbar")
            xbar_fm = early.tile([128, nd], F32, tag="xbarfm", name="xbarfm")
            xfh = early.tile([128, nd], F16, tag="xfh", name="xfh")
            xfh32 = early.tile([128, nd], F32, tag="xfh32", name="xfh32")
            xfl = early.tile([128, nd], F16, tag="xfl", name="xfl")
            S_sb = early.tile([1, 2 * R], F32, tag="Ssb", name="Ssb")
            noff_sb = early.tile([1, 1], F16, tag="noff", name="noff")
            noffb = early.tile([128, 1], F32, tag="noffb", name="noffb")
            inits = [
                early.tile([128, 1], F32, tag=f"init{q}", name=f"init{q}")
                for q in range(2 * nr)
            ]

            nc.vector.memset(ones_col[:, :], 1.0)
            nc.vector.memset(ones_row[:, :], 1.0)
            nc.vector.memset(id1[:, :], 1.0)

            # mask broadcast (rank-1 PE outer product), resident [128, TC] f16
            for t in range(nt):
                tsl = slice(t * TT, (t + 1) * TT)
                psm = pss.tile([128, TT], F32, tag="small", name="small")
                nc.tensor.matmul(
                    psm[:, :], ones_row[:, :], mrow_pre[:, tsl], start=True, stop=True
                )
                nc.vector.tensor_copy(masks[:, tsl], psm[:, :])

            # n carry from mprev alone (no xprev dependency), n-scan + scalers
            nred = early.tile([128, 1], F32, tag="nred", name="nred")
            nred16 = early.tile([128, 1], F16, tag="nred16", name="nred16")
            nc.vector.tensor_reduce(nred[:, :], mprev_pre[:, :], mybir.AxisListType.X, AL.add)
            nc.vector.tensor_copy(nred16[:, :], nred[:, :])
            ps_nf = pss.tile([1, 1], F32, tag="small", name="small")
            nc.tensor.matmul(ps_nf[:, :], nred16[:, :], ones_col[:, :], start=True, stop=True)
            nc.vector.tensor_copy(noff_sb[:, :], ps_nf[:, :])
            ps_nb = pss.tile([128, 1], F32, tag="small", name="small")
            nc.tensor.matmul(ps_nb[:, :], ones_row[:, :], noff_sb[:, :], start=True, stop=True)
            nc.vector.tensor_copy(noffb[:, :], ps_nb[:, :])
            for t in range(nt):
                tsl = slice(t * TT, (t + 1) * TT)
                init = noffb[:, :] if t == 0 else nb[:, t * TT - 1 : t * TT]
                nc.vector.tensor_tensor_scan(
                    nb[:, tsl], masks[:, tsl], masks[:, tsl], init, AL.add, AL.bypass
                )
            nc.vector.tensor_scalar_min(minn[:, :], nb[0:1, :], 1.0)
            for t in range(nt):
                tsl = slice(t * TT, (t + 1) * TT)
                nc.vector.tensor_scalar_max(nb[:, tsl], nb[:, tsl], 1.0)
                nc.vector.reciprocal(nb[:, tsl], nb[:, tsl])

            # K-side projections (Bk, C): 3-pass fp16 hi/lo + masked evac
            bks = {}
            for q in range(2 * nr):
                mcol = 2 * R + q * 128
                for t in range(nt):
                    tsl = slice(t * TT, (t + 1) * TT)
                    pt = psb.tile([128, TT], F32, tag="pt", name="pt")
                    first = True
                    for xs, ps in ((xtsh, pcsh), (xtsh, pcsl), (xtsl, pcsh)):
                        for d in range(nd):
                            nc.tensor.matmul(
                                pt[:, :], ps[d][:, mcol : mcol + 128], xs[d][:, tsl],
                                start=first,
                                stop=(xs is xtsl and d == nd - 1),
                            )
                            first = False
                    bk = bkp.tile([128, TT], F32, tag="bk", name="bk")
                    nc.vector.tensor_mul(bk[:, :], pt[:, :], masks[:, tsl])
                    bks[(q, t)] = bk

            # cross-half carry: xbar = mprev^T @ (xprevh + xprevl)
            n512 = (D + 511) // 512
            ps_xb = [
                pss.tile([1, min(512, D - j * 512)], F32, tag="small", name="small")
                for j in range(n512)
            ]
            assert ntc % 2 == 0
            for half, xsrc in enumerate((xprevh, xprevl)):
                for i2 in range(ntc // 2):
                    xp = xpp.tile([128, 2 * D], F16, tag="xprev", name="xprev")
                    src_ap = xsrc[i2 * 256 : (i2 + 1) * 256, :].rearrange(
                        "(c p) d -> p c d", p=128
                    )
                    nc.sync.dma_start(
                        xp[:, :].rearrange("p (c d) -> p c d", c=2), src_ap
                    )
                    touch(xp)
                    for c in range(2):
                        i = 2 * i2 + c
                        lhs = mprev_pre[:, i : i + 1]
                        for j in range(n512):
                            w = min(512, D - j * 512)
                            nc.tensor.matmul(
                                ps_xb[j][:, :], lhs,
                                xp[:, c * D + j * 512 : c * D + j * 512 + w],
                                start=(i == 0 and half == 0),
                                stop=(i == ntc - 1 and half == 1),
                            )
            for j in range(n512):
                w = min(512, D - j * 512)
                nc.vector.tensor_copy(xbar_sb[:, j * 512 : j * 512 + w], ps_xb[j][:, :])

            for j in range(nd):
                pst = pss.tile([128, 1], F32, tag="small", name="small")
                nc.tensor.transpose(
                    pst[:, :], xbar_sb[:, j * 128 : (j + 1) * 128], id1[:, :]
                )
                nc.vector.tensor_copy(xbar_fm[:, j : j + 1], pst[:, :])

            # split xbar into fp16 hi/lo, then S = xbar @ [P3|P4] (3-pass)
            nc.vector.tensor_copy(xfh[:, :], xbar_fm[:, :])
            nc.vector.tensor_copy(xfh32[:, :], xfh[:, :])
            nc.vector.tensor_sub(xfl[:, :], xbar_fm[:, :], xfh32[:, :])
            ps_S = pss.tile([1, 2 * R], F32, tag="small", name="small")
            for d in range(nd):
                ops = [(xfh, pcsh[d]), (xfh, pcsl[d]), (xfl, pcsh[d])]
                for k, (xo, po) in enumerate(ops):
                    nc.tensor.matmul(
                        ps_S[:, :], xo[:, d : d + 1], po[:, 2 * R : 4 * R],
                        start=(d == 0 and k == 0),
                        stop=(d == nd - 1 and k == len(ops) - 1),
                    )
            nc.vector.tensor_copy(S_sb[:, :], ps_S[:, :])

            for q in range(2 * nr):
                pst = pss.tile([128, 1], F32, tag="small", name="small")
                nc.tensor.transpose(
                    pst[:, :], S_sb[:, q * 128 : (q + 1) * 128], id1[:, :]
                )
                nc.vector.tensor_copy(inits[q][:, :], pst[:, :])

            # scans: local cumsum (initial=0) so they pipeline right behind the
            # projections; the cross-half carry is added afterwards as a
            # per-partition scalar (keeps scans off the xprev-stream path)
            for q in range(2 * nr):
                for t in range(nt):
                    tsl = slice(t * TT, (t + 1) * TT)
                    init = 0.0 if t == 0 else cums[q][:, t * TT - 1 : t * TT]
                    bk = bks[(q, t)]
                    nc.vector.tensor_tensor_scan(
                        cums[q][:, tsl], bk[:, :], bk[:, :], init, AL.add, AL.bypass
                    )
            for q in range(2 * nr):
                nc.vector.tensor_scalar_add(cums[q][:, :], cums[q][:, :], inits[q][:, :])

        # --- phase D: A/E projections, G, final matmul ---
        with ExitStack() as lctx:
            late = lctx.enter_context(tc.tile_pool(name="late", bufs=1))
            aep = lctx.enter_context(tc.tile_pool(name="aep", bufs=6))
            gp = lctx.enter_context(tc.tile_pool(name="gp", bufs=2))
            outp = lctx.enter_context(tc.tile_pool(name="outp", bufs=4))


            for t in range(nt):
                tsl = slice(t * TT, (t + 1) * TT)
                aes = []
                for m in range(2 * nr):
                    pa = psb.tile([128, TT], F32, tag="pt", name="pt")
                    first = True
                    for xs, ps in ((xtsh, pcsh), (xtsh, pcsl), (xtsl, pcsh)):
                        for d in range(nd):
                            nc.tensor.matmul(
                                pa[:, :], ps[d][:, m * 128 : (m + 1) * 128], xs[d][:, tsl],
                                start=first,
                                stop=(xs is xtsl and d == nd - 1),
                            )
                            first = False
                    ae = aep.tile([128, TT], F32, tag="ae", name="ae")
                    nc.scalar.copy(ae[:, :], pa[:, :])
                    aes.append(ae)
                ghs, gls = [], []
                for r in range(nr):
                    t1 = gp.tile([128, TT], F32, tag="g1", name="g1")
                    nc.vector.tensor_mul(t1[:, :], aes[r][:, :], cums[r][:, tsl])
                    t2 = gp.tile([128, TT], F32, tag="g2", name="g2")
                    nc.vector.tensor_mul(t2[:, :], aes[nr + r][:, :], cums[nr + r][:, tsl])
                    nc.vector.tensor_add(t1[:, :], t1[:, :], t2[:, :])
                    g = gp.tile([128, TT], F32, tag="g", name="g", bufs=4)
                    nc.vector.tensor_mul(g[:, :], t1[:, :], nb[:, tsl])
                    gh = gp.tile([128, TT], F16, tag="gh", name="gh", bufs=4)
                    nc.scalar.copy(gh[:, :], g[:, :])
                    gl = gp.tile([128, TT], F16, tag="gl", name="gl", bufs=4)
                    nc.vector.tensor_sub(gl[:, :], g[:, :], gh[:, :])
                    ghs.append(gh)
                    gls.append(gl)
                for d in range(nd):
                    po = psb.tile([128, TT], F32, tag="pt", name="pt")
                    dsl = slice(d * 128, (d + 1) * 128)
                    first = True
                    for r in range(nr):
                        for zo, go in (
                            (ztsh[r], ghs[r]),
                            (ztsl[r], ghs[r]),
                            (ztsh[r], gls[r]),
                        ):
                            nc.tensor.matmul(
                                po[:, :], zo[:, dsl], go[:, :], start=first, stop=False
                            )
                            first = False
                    nc.tensor.matmul(
                        po[:, :], bvec_sb[:, d * 128 : (d + 1) * 128], minn[:, tsl],
                        start=False, stop=True,
                    )
                    ot = outp.tile([128, TT], F32, tag="ot", name="ot")
                    nc.scalar.copy(ot[:, :], po[:, :])
                    nc.sync.dma_start(outT[d * 128 : (d + 1) * 128, tsl], ot[:, :])

    nc.finalize()
    if hoist:
        split_excess_waits(nc)
    return nc


def make_core_inputs(x, attention_mask, Pcat, ZT, bvec):
    B, T, D = x.shape
    TC = T // N_SEQ_SHARDS
    m = np.asarray(attention_mask).astype(np.float16)
    Ph, Pl = _split16(Pcat)
    Zh, Zl = _split16(ZT)
    bv16 = bvec.astype(np.float16)
    in_maps = []
    for b in range(B):
        for h in range(N_SEQ_SHARDS):
            sl = slice(h * TC, (h + 1) * TC)
            psl = slice((h - 1) * TC, h * TC) if h > 0 else slice(0, TC)
            mp = m[b, psl] if h > 0 else np.zeros(TC, np.float16)
            xT = np.ascontiguousarray(x[b, sl, :].T)
            xTh, xTl = _split16(xT)
            xprevh, xprevl = _split16(x[b, psl, :])
            in_maps.append(
                {
                    "xTh": xTh,
                    "xTl": xTl,
                    "xprevh": np.ascontiguousarray(xprevh),
                    "xprevl": np.ascontiguousarray(xprevl),
                    "mrow": np.ascontiguousarray(m[b, sl])[None, :],
                    "mprev": np.ascontiguousarray(mp.reshape(TC // 128, 128).T),
                    "Pcath": Ph,
                    "Pcatl": Pl,
                    "ZTh": Zh,
                    "ZTl": Zl,
                    "bvec": bv16,
                }
            )
    return in_maps


_NC_CACHE = {}


def get_nc(D, TC, R):
    key = (D, TC, R)
    if key not in _NC_CACHE:
        _NC_CACHE[key] = build_nc(D, TC, R)
    return _NC_CACHE[key]


def kernel(x, Wq, Wk, Wo, Winv, U, V, Wm, bias, alpha, attention_mask):
    x = np.asarray(x, np.float32)
    B, T, D = x.shape
    R = np.asarray(U).shape[1]
    TC = T // N_SEQ_SHARDS
    Pcat, ZT, bvec = fold_weights(Wq, Wk, Wo, Winv, U, V, Wm, bias, alpha)
    nc = get_nc(D, TC, R)
    in_maps = make_core_inputs(x, np.asarray(attention_mask), Pcat, ZT, bvec)
    res = run_bass_kernel_spmd(nc, in_maps, core_ids=list(range(N_CORES)))
    out = np.empty((B, T, D), np.float32)
    k = 0
    for b in range(B):
        for h in range(N_SEQ_SHARDS):
            out[b, h * TC : (h + 1) * TC, :] = res.results[k]["outT"].T
            k += 1
    return out


# revision 18
# speedup vs baseline: 1.0564x; 1.0039x over previous
"""Trainium2 Bass kernel for nn_CausalBiBCNAttention (B=4, T=4096, D=1024, R=256).

Algebra (exact rewrite of the reference):
    out = G @ (Wo@U).T + min(n,1)*(1+alpha)*(Wo@bias)
    G   = (A*cumsum(Bk) + E*cumsum(C)) / max(n,1)
    A   = x @ (Wq.T V);  E = x @ (Wq.T Winv.T Wm)
    Bk  = (x @ (Wk.T Wm)) * m;  C = alpha * (x @ (Wk.T Winv.T V)) * m
    n   = cumsum(m)
The five DxD projections fold into four DxR matrices (host constant folding in
f64); the device does 5 rank-R projections + DVE prefix-scans (cumsum maps to
the native tensor_tensor_scan along the free axis).

Precision: matmul operands are fp16 hi/lo pairs (x = xh + xl exactly to
~2^-22); each contraction runs 3 passes (xh*Ph + xh*Pl + xl*Ph, the xl*Pl
term is ~2^-22 and dropped) accumulated in fp32 PSUM. fp16 streams at
1 col/cycle on the PE (fp32 runs 2 half-rate passes = 4x slower).

Sharding: 8 cores = batch(4) x sequence-halves(2). The cumsum carry for the
second half is computed on-device from a mask-weighted reduction of the
previous half (xbar = mprev^T @ xprev; S = xbar @ [P3|P4]).
"""

from contextlib import ExitStack

import numpy as np

import concourse.bass as bass
import concourse.mybir as mybir
import concourse.tile as tile
from concourse.bass_utils import run_bass_kernel_spmd

F32 = mybir.dt.float32
F16 = mybir.dt.float16
AL = mybir.AluOpType

N_CORES = 8
N_SEQ_SHARDS = 2


def _split16(a):
    """fp16 hi/lo pair: a ~= hi + lo with ~2^-22 relative residual."""
    hi = a.astype(np.float16)
    lo = (a - hi.astype(np.float32)).astype(np.float16)
    return hi, lo


def fold_weights(Wq, Wk, Wo, Winv, U, V, Wm, bias, alpha):
    Wq, Wk, Wo, Winv, U, V, Wm, bias = (
        np.asarray(a, np.float64) for a in (Wq, Wk, Wo, Winv, U, V, Wm, bias)
    )
    alpha = float(alpha)
    P1 = Wq.T @ V
    P2 = Wq.T @ Winv.T @ Wm
    P3 = Wk.T @ Wm
    P4 = alpha * (Wk.T @ (Winv.T @ V))
    Pcat = np.concatenate([P1, P2, P3, P4], axis=1).astype(np.float32)
    ZT = np.ascontiguousarray((Wo @ U).T).astype(np.float32)
    bvec = ((1.0 + alpha) * (Wo @ bias)).astype(np.float32)[None, :]
    return Pcat, ZT, bvec


def split_excess_waits(nc, max_waits=1):
    """Hoist excess per-instruction sync waits onto preceding same-engine NoOps.

    Walrus's per-instruction sync budget rejects >1 wait command on several
    instruction structs (fp32 Matmult, DMA pseudo-ops). Engine streams execute
    in order, so a NoOp carrying the extra wait immediately before the
    instruction is semantically identical.
    """
    fn = nc.m.functions[0]
    k = 0
    for blk in fn.blocks:
        new_insts = []
        for ins in blk.instructions:
            si = getattr(ins, "sync_info", None)
            if si is not None and si.on_wait and len(si.on_wait) > max_waits:
                waits = list(si.on_wait)
                for w in waits[:-max_waits]:
                    k += 1
                    new_insts.append(
                        mybir.InstNoOp(
                            name=f"{ins.name}-hoistw{k}",
                            engine=ins.engine,
                            ins=[],
                            outs=[],
                            sync_info=mybir.SyncInfo(on_wait=[w], on_update=[]),
                            bass_nofuse=True,
                        )
                    )
                ins.sync_info = mybir.SyncInfo(
                    on_wait=waits[-max_waits:], on_update=si.on_update
                )
            new_insts.append(ins)
        blk.instructions[:] = new_insts
    return nc


def build_nc(D, TC, R, TT=512, hoist=True):
    assert D % 128 == 0 and R % 128 == 0 and TC % TT == 0 and TC % 128 == 0
    nd, nr, nt, ntc = D // 128, R // 128, TC // TT, TC // 128

    nc = bass.Bass()
    xTh = nc.dram_tensor("xTh", (D, TC), F16, kind="ExternalInput")
    xTl = nc.dram_tensor("xTl", (D, TC), F16, kind="ExternalInput")
    xprevh = nc.dram_tensor("xprevh", (TC, D), F16, kind="ExternalInput")
    xprevl = nc.dram_tensor("xprevl", (TC, D), F16, kind="ExternalInput")
    mrow = nc.dram_tensor("mrow", (1, TC), F16, kind="ExternalInput")
    mprev = nc.dram_tensor("mprev", (128, ntc), F16, kind="ExternalInput")
    Pcath = nc.dram_tensor("Pcath", (D, 4 * R), F16, kind="ExternalInput")
    Pcatl = nc.dram_tensor("Pcatl", (D, 4 * R), F16, kind="ExternalInput")
    ZTh = nc.dram_tensor("ZTh", (R, D), F16, kind="ExternalInput")
    ZTl = nc.dram_tensor("ZTl", (R, D), F16, kind="ExternalInput")
    bvecd = nc.dram_tensor("bvec", (1, D), F16, kind="ExternalInput")
    outT = nc.dram_tensor("outT", (D, TC), F32, kind="ExternalOutput")

    with tile.TileContext(nc) as tc, ExitStack() as ctx:
        res = ctx.enter_context(tc.tile_pool(name="res", bufs=1))
        psb = ctx.enter_context(tc.tile_pool(name="psb", bufs=4, space="PSUM"))
        pss = ctx.enter_context(tc.tile_pool(name="pss", bufs=3, space="PSUM"))
        psj = ctx.enter_context(tc.tile_pool(name="psj", bufs=1, space="PSUM"))
        junk = psj.tile([1, 1], F32, tag="junk", name="junk")

        def touch(t):
            # absorb the tile's DMA-completion wait into a 1-element PE matmul
            # (several instruction structs carry at most ONE sync wait; this
            # keeps every real matmul's unsatisfied-dependency count at <= 1)
            k = min(t.shape[0], 128)
            a = t[0:k, 0:1]
            nc.tensor.matmul(junk[:, :], a, a, start=True, stop=True)

        # --- resident tiles ---
        xtsh = [res.tile([128, TC], F16, tag=f"xth{d}", name=f"xth{d}") for d in range(nd)]
        xtsl = [res.tile([128, TC], F16, tag=f"xtl{d}", name=f"xtl{d}") for d in range(nd)]
        pcsh = [res.tile([128, 4 * R], F16, tag=f"pch{d}", name=f"pch{d}") for d in range(nd)]
        pcsl = [res.tile([128, 4 * R], F16, tag=f"pcl{d}", name=f"pcl{d}") for d in range(nd)]
        ztsh = [res.tile([128, D], F16, tag=f"zth{r}", name=f"zth{r}") for r in range(nr)]
        ztsl = [res.tile([128, D], F16, tag=f"ztl{r}", name=f"ztl{r}") for r in range(nr)]
        cums = [res.tile([128, TC], F32, tag=f"cum{q}", name=f"cum{q}") for q in range(2 * nr)]
        nb = res.tile([128, TC], F32, tag="nb", name="nb")
        minn = res.tile([1, TC], F16, tag="minn", name="minn")
        bvec_sb = res.tile([1, D], F16, tag="bvec", name="bvec")

        mrow_pre = res.tile([1, TC], F16, tag="mrowp", name="mrowp")
        mprev_pre = res.tile([128, ntc], F16, tag="mprevp", name="mprevp")
        nc.sync.dma_start(mrow_pre[:, :], mrow[:, :])
        touch(mrow_pre)
        nc.sync.dma_start(mprev_pre[:, :], mprev[:, :])
        touch(mprev_pre)
        for d in range(nd):
            nc.sync.dma_start(pcsh[d][:, :], Pcath[d * 128 : (d + 1) * 128, :])
            touch(pcsh[d])
            nc.sync.dma_start(xtsh[d][:, :], xTh[d * 128 : (d + 1) * 128, :])
            touch(xtsh[d])
        for d in range(nd):
            nc.sync.dma_start(pcsl[d][:, :], Pcatl[d * 128 : (d + 1) * 128, :])
            touch(pcsl[d])
            nc.sync.dma_start(xtsl[d][:, :], xTl[d * 128 : (d + 1) * 128, :])
            touch(xtsl[d])
        for r in range(nr):
            nc.sync.dma_start(ztsh[r][:, :], ZTh[r * 128 : (r + 1) * 128, :])
            touch(ztsh[r])
            nc.sync.dma_start(ztsl[r][:, :], ZTl[r * 128 : (r + 1) * 128, :])
            touch(ztsl[r])
        nc.sync.dma_start(bvec_sb[:, :], bvecd[:, :])
        touch(bvec_sb)

        with ExitStack() as ectx:
            early = ectx.enter_context(tc.tile_pool(name="early", bufs=1))
            xpp = ectx.enter_context(tc.tile_pool(name="xpp", bufs=2))
            bkp = ectx.enter_context(tc.tile_pool(name="bkp", bufs=4))

            masks = early.tile([128, TC], F16, tag="masks", name="masks")
            ones_col = early.tile([128, 1], F16, tag="ones_col", name="ones_col")
            ones_row = early.tile([1, 128], F16, tag="ones_row", name="ones_row")
            id1 = early.tile([1, 1], F32, tag="id1", name="id1")
            xbar_sb = early.tile([1, D], F32, tag="xbar", name="xbar")
            xbar_fm = early.tile([128, nd], F32, tag="xbarfm", name="xbarfm")
            xfh = early.tile([128, nd], F16, tag="xfh", name="xfh")
            xfh32 = early.tile([128, nd], F32, tag="xfh32", name="xfh32")
            xfl = early.tile([128, nd], F16, tag="xfl", name="xfl")
            S_sb = early.tile([1, 2 * R], F32, tag="Ssb", name="Ssb")
            noff_sb = early.tile([1, 1], F16, tag="noff", name="noff")
            noffb = early.tile([128, 1], F32, tag="noffb", name="noffb")
            inits = [
                early.tile([128, 1], F32, tag=f"init{q}", name=f"init{q}")
                for q in range(2 * nr)
            ]

            nc.vector.memset(ones_col[:, :], 1.0)
            nc.vector.memset(ones_row[:, :], 1.0)
            nc.vector.memset(id1[:, :], 1.0)

            # mask broadcast (rank-1 PE outer product), resident [128, TC] f16
            for t in range(nt):
                tsl = slice(t * TT, (t + 1) * TT)
                psm = pss.tile([128, TT], F32, tag="small", name="small")
                nc.tensor.matmul(
                    psm[:, :], ones_row[:, :], mrow_pre[:, tsl], start=True, stop=True
                )
                nc.vector.tensor_copy(masks[:, tsl], psm[:, :])

            # n carry from mprev alone (no xprev dependency), n-scan + scalers
            nred = early.tile([128, 1], F32, tag="nred", name="nred")
            nred16 = early.tile([128, 1], F16, tag="nred16", name="nred16")
            nc.vector.tensor_reduce(nred[:, :], mprev_pre[:, :], mybir.AxisListType.X, AL.add)
            nc.vector.tensor_copy(nred16[:, :], nred[:, :])
            ps_nf = pss.tile([1, 1], F32, tag="small", name="small")
            nc.tensor.matmul(ps_nf[:, :], nred16[:, :], ones_col[:, :], start=True, stop=True)
            nc.vector.tensor_copy(noff_sb[:, :], ps_nf[:, :])
            ps_nb = pss.tile([128, 1], F32, tag="small", name="small")
            nc.tensor.matmul(ps_nb[:, :], ones_row[:, :], noff_sb[:, :], start=True, stop=True)
            nc.vector.tensor_copy(noffb[:, :], ps_nb[:, :])
            for t in range(nt):
                tsl = slice(t * TT, (t + 1) * TT)
                init = noffb[:, :] if t == 0 else nb[:, t * TT - 1 : t * TT]
                nc.vector.tensor_tensor_scan(
                    nb[:, tsl], masks[:, tsl], masks[:, tsl], init, AL.add, AL.bypass
                )
            nc.vector.tensor_scalar_min(minn[:, :], nb[0:1, :], 1.0)
            for t in range(nt):
                tsl = slice(t * TT, (t + 1) * TT)
                nc.vector.tensor_scalar_max(nb[:, tsl], nb[:, tsl], 1.0)
                nc.vector.reciprocal(nb[:, tsl], nb[:, tsl])

            # K-side projections (Bk, C): 3-pass fp16 hi/lo + masked evac
            bks = {}
            for q in range(2 * nr):
                mcol = 2 * R + q * 128
                for t in range(nt):
                    tsl = slice(t * TT, (t + 1) * TT)
                    pt = psb.tile([128, TT], F32, tag="pt", name="pt")
                    first = True
                    for xs, ps in ((xtsh, pcsh), (xtsh, pcsl), (xtsl, pcsh)):
                        for d in range(nd):
                            nc.tensor.matmul(
                                pt[:, :], ps[d][:, mcol : mcol + 128], xs[d][:, tsl],
                                start=first,
                                stop=(xs is xtsl and d == nd - 1),
                            )
                            first = False
                    bk = bkp.tile([128, TT], F32, tag="bk", name="bk")
                    nc.vector.tensor_mul(bk[:, :], pt[:, :], masks[:, tsl])
                    bks[(q, t)] = bk

            # cross-half carry: xbar = mprev^T @ (xprevh + xprevl)
            n512 = (D + 511) // 512
            ps_xb = [
                pss.tile([1, min(512, D - j * 512)], F32, tag="small", name="small")
                for j in range(n512)
            ]
            assert ntc % 2 == 0
            for half, xsrc in enumerate((xprevh, xprevl)):
                for i2 in range(ntc // 2):
                    xp = xpp.tile([128, 2 * D], F16, tag="xprev", name="xprev")
                    src_ap = xsrc[i2 * 256 : (i2 + 1) * 256, :].rearrange(
                        "(c p) d -> p c d", p=128
                    )
                    nc.sync.dma_start(
                        xp[:, :].rearrange("p (c d) -> p c d", c=2), src_ap
                    )
                    touch(xp)
                    for c in range(2):
                        i = 2 * i2 + c
                        lhs = mprev_pre[:, i : i + 1]
                        for j in range(n512):
                            w = min(512, D - j * 512)
                            nc.tensor.matmul(
                                ps_xb[j][:, :], lhs,
                                xp[:, c * D + j * 512 : c * D + j * 512 + w],
                                start=(i == 0 and half == 0),
                                stop=(i == ntc - 1 and half == 1),
                            )
            for j in range(n512):
                w = min(512, D - j * 512)
                nc.vector.tensor_copy(xbar_sb[:, j * 512 : j * 512 + w], ps_xb[j][:, :])

            for j in range(nd):
                pst = pss.tile([128, 1], F32, tag="small", name="small")
                nc.tensor.transpose(
                    pst[:, :], xbar_sb[:, j * 128 : (j + 1) * 128], id1[:, :]
                )
                nc.vector.tensor_copy(xbar_fm[:, j : j + 1], pst[:, :])

            # split xbar into fp16 hi/lo, then S = xbar @ [P3|P4] (3-pass)
            nc.vector.tensor_copy(xfh[:, :], xbar_fm[:, :])
            nc.vector.tensor_copy(xfh32[:, :], xfh[:, :])
            nc.vector.tensor_sub(xfl[:, :], xbar_fm[:, :], xfh32[:, :])
            ps_S = pss.tile([1, 2 * R], F32, tag="small", name="small")
            for d in range(nd):
                ops = [(xfh, pcsh[d]), (xfh, pcsl[d]), (xfl, pcsh[d])]
                for k, (xo, po) in enumerate(ops):
                    nc.tensor.matmul(
                        ps_S[:, :], xo[:, d : d + 1], po[:, 2 * R : 4 * R],
                        start=(d == 0 and k == 0),
                        stop=(d == nd - 1 and k == len(ops) - 1),
                    )
            nc.vector.tensor_copy(S_sb[:, :], ps_S[:, :])

            for q in range(2 * nr):
                pst = pss.tile([128, 1], F32, tag="small", name="small")
                nc.tensor.transpose(
                    pst[:, :], S_sb[:, q * 128 : (q + 1) * 128], id1[:, :]
                )
                nc.vector.tensor_copy(inits[q][:, :], pst[:, :])

            # scans: local cumsum (initial=0) so they pipeline right behind the
            # projections; the cross-half carry is added afterwards as a
            # per-partition scalar (keeps scans off the xprev-stream path)
            for q in range(2 * nr):
                for t in range(nt):
                    tsl = slice(t * TT, (t + 1) * TT)
                    init = 0.0 if t == 0 else cums[q][:, t * TT - 1 : t * TT]
                    bk = bks[(q, t)]
                    nc.vector.tensor_tensor_scan(
                        cums[q][:, tsl], bk[:, :], bk[:, :], init, AL.add, AL.bypass
                    )
            for q in range(2 * nr):
                nc.vector.tensor_scalar_add(cums[q][:, :], cums[q][:, :], inits[q][:, :])

        # --- phase D: A/E projections, G, final matmul ---
        with ExitStack() as lctx:
            late = lctx.enter_context(tc.tile_pool(name="late", bufs=1))
            aep = lctx.enter_context(tc.tile_pool(name="aep", bufs=6))
            gp = lctx.enter_context(tc.tile_pool(name="gp", bufs=2))
            outp = lctx.enter_context(tc.tile_pool(name="outp", bufs=4))


            for t in range(nt):
                tsl = slice(t * TT, (t + 1) * TT)
                aes = []
                for m in range(2 * nr):
                    pa = psb.tile([128, TT], F32, tag="pt", name="pt")
                    first = True
                    for xs, ps in ((xtsh, pcsh), (xtsh, pcsl), (xtsl, pcsh)):
                        for d in range(nd):
                            nc.tensor.matmul(
                                pa[:, :], ps[d][:, m * 128 : (m + 1) * 128], xs[d][:, tsl],
                                start=first,
                                stop=(xs is xtsl and d == nd - 1),
                            )
                            first = False
                    ae = aep.tile([128, TT], F32, tag="ae", name="ae")
                    nc.scalar.copy(ae[:, :], pa[:, :])
                    aes.append(ae)
                ghs, gls = [], []
                for r in range(nr):
                    t1 = gp.tile([128, TT], F32, tag="g1", name="g1")
                    nc.vector.tensor_mul(t1[:, :], aes[r][:, :], cums[r][:, tsl])
                    t2 = gp.tile([128, TT], F32, tag="g2", name="g2")
                    nc.vector.tensor_mul(t2[:, :], aes[nr + r][:, :], cums[nr + r][:, tsl])
                    nc.vector.tensor_add(t1[:, :], t1[:, :], t2[:, :])
                    g = gp.tile([128, TT], F32, tag="g", name="g", bufs=4)
                    nc.vector.tensor_mul(g[:, :], t1[:, :], nb[:, tsl])
                    gh = gp.tile([128, TT], F16, tag="gh", name="gh", bufs=4)
                    nc.scalar.copy(gh[:, :], g[:, :])
                    gl = gp.tile([128, TT], F16, tag="gl", name="gl", bufs=4)
                    nc.vector.tensor_sub(gl[:, :], g[:, :], gh[:, :])
                    ghs.append(gh)
                    gls.append(gl)
                for d in range(nd):
                    po = psb.tile([128, TT], F32, tag="pt", name="pt")
                    dsl = slice(d * 128, (d + 1) * 128)
                    first = True
                    for r in range(nr):
                        for zo, go in (
                            (ztsh[r], ghs[r]),
                            (ztsl[r], ghs[r]),
                            (ztsh[r], gls[r]),
                        ):
                            nc.tensor.matmul(
                                po[:, :], zo[:, dsl], go[:, :], start=first, stop=False
                            )
                            first = False
                    nc.tensor.matmul(
                        po[:, :], bvec_sb[:, d * 128 : (d + 1) * 128], minn[:, tsl],
                        start=False, stop=True,
                    )
                    ot = outp.tile([128, TT], F32, tag="ot", name="ot")
                    nc.scalar.copy(ot[:, :], po[:, :])
                    nc.sync.dma_start(outT[d * 128 : (d + 1) * 128, tsl], ot[:, :])

    nc.finalize()
    if hoist:
        split_excess_waits(nc)
    return nc


def make_core_inputs(x, attention_mask, Pcat, ZT, bvec):
    B, T, D = x.shape
    TC = T // N_SEQ_SHARDS
    m = np.asarray(attention_mask).astype(np.float16)
    Ph, Pl = _split16(Pcat)
    Zh, Zl = _split16(ZT)
    bv16 = bvec.astype(np.float16)
    in_maps = []
    for b in range(B):
        for h in range(N_SEQ_SHARDS):
            sl = slice(h * TC, (h + 1) * TC)
            psl = slice((h - 1) * TC, h * TC) if h > 0 else slice(0, TC)
            mp = m[b, psl] if h > 0 else np.zeros(TC, np.float16)
            xT = np.ascontiguousarray(x[b, sl, :].T)
            xTh, xTl = _split16(xT)
            xprevh, xprevl = _split16(x[b, psl, :])
            in_maps.append(
                {
                    "xTh": xTh,
                    "xTl": xTl,
                    "xprevh": np.ascontiguousarray(xprevh),
                    "xprevl": np.ascontiguousarray(xprevl),
                    "mrow": np.ascontiguousarray(m[b, sl])[None, :],
                    "mprev": np.ascontiguousarray(mp.reshape(TC // 128, 128).T),
                    "Pcath": Ph,
                    "Pcatl": Pl,
                    "ZTh": Zh,
                    "ZTl": Zl,
                    "bvec": bv16,
                }
            )
    return in_maps


_NC_CACHE = {}


def get_nc(D, TC, R):
    key = (D, TC, R)
    if key not in _NC_CACHE:
        _NC_CACHE[key] = build_nc(D, TC, R)
    return _NC_CACHE[key]


def kernel(x, Wq, Wk, Wo, Winv, U, V, Wm, bias, alpha, attention_mask):
    x = np.asarray(x, np.float32)
    B, T, D = x.shape
    R = np.asarray(U).shape[1]
    TC = T // N_SEQ_SHARDS
    Pcat, ZT, bvec = fold_weights(Wq, Wk, Wo, Winv, U, V, Wm, bias, alpha)
    nc = get_nc(D, TC, R)
    in_maps = make_core_inputs(x, np.asarray(attention_mask), Pcat, ZT, bvec)
    res = run_bass_kernel_spmd(nc, in_maps, core_ids=list(range(N_CORES)))
    out = np.empty((B, T, D), np.float32)
    k = 0
    for b in range(B):
        for h in range(N_SEQ_SHARDS):
            out[b, h * TC : (h + 1) * TC, :] = res.results[k]["outT"].T
            k += 1
    return out
